# revision 5
# baseline (speedup 1.0000x reference)
"""Multi-head causal attention (B=4, S=2048, D=1024, H=16) on 8 TRN2 NeuronCores.

Sharding: core c -> (batch b = c//2, head-half hh = c%2). Each core computes
8 heads (a 512-wide feature slice) for one batch element, including its
partial W_o projection; the host sums the two partials per batch.

Per-core kernel (all matmuls float32r, full-rate at N>=256):
  phase 1: PE-transpose key/value s-blocks to feature-major, project to
           K^T [512,2048] (feature-major) and V_aug [2048, 8x(64+1)] (natural,
           with a ones column per head for the softmax denominator).
  phase 2 (per 512-wide q-block): transpose+project Q^T; per head, scores^T
           tiles [k=128, q=512] via QK^T (2 heads row-packed in the PE array),
           causal mask on diagonal squares, exp on ACT (scale=1/8),
           PV matmul with lhsT=[V|1] accumulating unnormalized out^T and the
           denominator row in one PSUM tile; normalize via DVE reciprocal +
           K=1 ones-broadcast matmul; W_o partial projection; DMA out.
"""

import numpy as np

import concourse.bass as bass
import concourse.mybir as mybir
import concourse.tile as tile
from concourse import bacc
from concourse.bass_utils import run_bass_kernel_spmd

P = 128
FR = mybir.dt.float32r
F32 = mybir.dt.float32
AF = mybir.ActivationFunctionType
OP = mybir.AluOpType

B, S, D, H = 4, 2048, 1024, 16
DK = D // H            # 64
DH = D // 2            # 512: per-core feature slice (8 heads)
NH = DH // DK          # 8 heads per core
QB = 512               # q-block width
NJ = S // QB           # 4 q-blocks
NST = S // P           # 16 s-tiles
NDT = D // P           # 8 din tiles
NOT_ = DH // P         # 4 dout tiles (per-core slice)
MASK_VAL = -1e30


def build_nc(loop_n=None):
    nc = bacc.Bacc(None, target_bir_lowering=False)

    xq = nc.dram_tensor("xq", [S, D], FR, kind="ExternalInput")
    xk = nc.dram_tensor("xk", [S, D], FR, kind="ExternalInput")
    xv = nc.dram_tensor("xv", [S, D], FR, kind="ExternalInput")
    wqT = nc.dram_tensor("wqT", [D, DH], FR, kind="ExternalInput")
    wkT = nc.dram_tensor("wkT", [D, DH], FR, kind="ExternalInput")
    wvT = nc.dram_tensor("wvT", [D, DH], FR, kind="ExternalInput")
    woT = nc.dram_tensor("woT", [DH, D], FR, kind="ExternalInput")
    bq = nc.dram_tensor("bq", [DH], F32, kind="ExternalInput")
    bk = nc.dram_tensor("bk", [DH], F32, kind="ExternalInput")
    bvb = nc.dram_tensor("bvb", [P, DH], F32, kind="ExternalInput")
    bob = nc.dram_tensor("bob", [P, D], F32, kind="ExternalInput")
    ident_d = nc.dram_tensor("ident", [P, P], FR, kind="ExternalInput")
    maskd_d = nc.dram_tensor("maskd", [P, P], F32, kind="ExternalInput")
    ones_d = nc.dram_tensor("ones", [P, DK], FR, kind="ExternalInput")
    out = nc.dram_tensor("out", [S, D], F32, kind="ExternalOutput")

    with tile.TileContext(nc) as tc:
        with (
            tc.tile_pool(name="cst", bufs=1) as cst,
            tc.tile_pool(name="wt", bufs=2) as wtp,
            tc.tile_pool(name="big", bufs=1) as big,
            tc.tile_pool(name="xT", bufs=2) as xTp,
            tc.tile_pool(name="xnat", bufs=5) as xnp,
            tc.tile_pool(name="qt", bufs=1) as qtp,
            tc.tile_pool(name="probs", bufs=2) as prp,
            tc.tile_pool(name="onrm", bufs=1) as onp,
            tc.tile_pool(name="yout", bufs=2) as yop,
            tc.tile_pool(name="rc", bufs=2) as rcp,
            tc.tile_pool(name="rb", bufs=2) as rbp,
            tc.tile_pool(name="ps_s", bufs=2, space="PSUM") as ps_s,
            tc.tile_pool(name="ps_o", bufs=2, space="PSUM") as ps_o,
            tc.tile_pool(name="ps_x", bufs=2, space="PSUM") as ps_x,
        ):
            def body():
                ident = cst.tile([P, P], FR, tag="ident")
                nc.sync.dma_start(ident[:], ident_d[:])
                maskd = cst.tile([P, P], F32, tag="maskd")
                nc.sync.dma_start(maskd[:], maskd_d[:])
                ones_sb = cst.tile([P, DK], FR, tag="ones")
                nc.sync.dma_start(ones_sb[:], ones_d[:])
                bq_sb = cst.tile([P, NOT_], F32, tag="bq")
                nc.sync.dma_start(bq_sb[:], bq.rearrange("(o p) -> p o", p=P))
                bk_sb = cst.tile([P, NOT_], F32, tag="bk")
                nc.sync.dma_start(bk_sb[:], bk.rearrange("(o p) -> p o", p=P))
                bvb_sb = cst.tile([P, DH], F32, tag="bvb")
                nc.sync.dma_start(bvb_sb[:], bvb[:])
                bob_sb = cst.tile([P, D], F32, tag="bob")
                nc.sync.dma_start(bob_sb[:], bob[:])

                wk_sb = wtp.tile([P, NDT, DH], FR, tag="wt")
                nc.sync.dma_start(wk_sb[:], wkT.rearrange("(o p) f -> p o f", p=P))
                wv_sb = wtp.tile([P, NDT, DH], FR, tag="wt")
                nc.sync.dma_start(wv_sb[:], wvT.rearrange("(o p) f -> p o f", p=P))

                # persistent: K^T (feature-major) and V_aug (natural + ones col)
                KT = big.tile([P, NOT_, S], FR, tag="KT")          # 32KB/part
                VA = big.tile([P, NST, NH * (DK + 1)], FR, tag="VA")  # 32.5KB/part

                def transpose_block(x_dram, sb):
                    """DMA s-block sb of x (natural) and PE-transpose to
                    feature-major xt [128, NDT, 512]. Returns the xT tile."""
                    xt = xTp.tile([P, NDT, QB], FR, tag="xT")
                    nats = []
                    for st in range(4):
                        xn = xnp.tile([P, D], FR, tag="xnat")
                        nc.sync.dma_start(
                            xn[:], x_dram[sb * QB + st * P: sb * QB + (st + 1) * P, :])
                        nats.append(xn)
                    for dt_i in range(NDT):
                        pt = ps_x.tile([P, QB], FR, tag="ps_x")
                        for st in range(4):
                            nc.tensor.transpose(
                                pt[:, st * P:(st + 1) * P],
                                nats[st][:, dt_i * P:(dt_i + 1) * P], ident[:])
                        nc.scalar.copy(xt[:, dt_i, :], pt[:])
                    return xt

                # ---- phase 1: K^T and V_aug ----
                for sb in range(NJ):
                    kt_x = transpose_block(xk, sb)
                    for ot in range(NOT_):
                        pk = ps_x.tile([P, QB], F32, tag="ps_x")
                        for dt_i in range(NDT):
                            nc.tensor.matmul(
                                pk[:], wk_sb[:, dt_i, ot * P:(ot + 1) * P],
                                kt_x[:, dt_i, :],
                                start=(dt_i == 0), stop=(dt_i == NDT - 1))
                        nc.scalar.activation(
                            KT[:, ot, sb * QB:(sb + 1) * QB], pk[:],
                            AF.Identity, bias=bk_sb[:, ot:ot + 1])
                    vt_x = transpose_block(xv, sb)
                    for st in range(4):
                        stg = sb * 4 + st
                        pv = ps_x.tile([P, QB], F32, tag="ps_x")
                        for dt_i in range(NDT):
                            nc.tensor.matmul(
                                pv[:], vt_x[:, dt_i, st * P:(st + 1) * P],
                                wv_sb[:, dt_i, :],
                                start=(dt_i == 0), stop=(dt_i == NDT - 1))
                        va_row = VA[:, stg, :].rearrange("p (h e) -> p h e", e=DK + 1)
                        nc.vector.tensor_tensor(
                            va_row[:, :, 0:DK],
                            pv[:].rearrange("p (h e) -> p h e", e=DK),
                            bvb_sb[:].rearrange("p (h e) -> p h e", e=DK),
                            OP.add)
                        nc.sync.dma_start(va_row[:, :, DK], ones_d[:, 0:NH])

                wq_sb = wtp.tile([P, NDT, DH], FR, tag="wt")
                nc.sync.dma_start(wq_sb[:], wqT.rearrange("(o p) f -> p o f", p=P))
                wo_sb = wtp.tile([P, NOT_, D], FR, tag="wt")
                nc.sync.dma_start(wo_sb[:], woT.rearrange("(o p) f -> p o f", p=P))

                # ---- phase 2: per q-block ----
                for j in range(NJ):
                    qt_x = transpose_block(xq, j)
                    QT = qtp.tile([P, NOT_, QB], FR, tag="qt")
                    for ot in range(NOT_):
                        pq = ps_x.tile([P, QB], F32, tag="ps_x")
                        for dt_i in range(NDT):
                            nc.tensor.matmul(
                                pq[:], wq_sb[:, dt_i, ot * P:(ot + 1) * P],
                                qt_x[:, dt_i, :],
                                start=(dt_i == 0), stop=(dt_i == NDT - 1))
                        nc.vector.tensor_scalar_add(
                            QT[:, ot, :], pq[:], bq_sb[:, ot:ot + 1])

                    ON = onp.tile([P, NOT_, QB], FR, tag="onrm")
                    nt = 4 * j + 4          # k-tiles for this q-block
                    for hp in range(NOT_):
                        po = [ps_o.tile([P, QB], F32, tag="ps_o", name=f"po{_i}") for _i in range(2)]
                        for tp in range(nt // 2):
                            sc = [ps_s.tile([P, 2 * QB], F32, tag="ps_s", name=f"sc{_i}")
                                  for _i in range(2)]
                            fs = []
                            for half in range(2):
                                th = 2 * tp + half
                                fstart = max(0, P * (th - 4 * j))
                                fs.append(fstart)
                                for hr in range(2):
                                    b0 = hr * DK
                                    nc.tensor.matmul(
                                        sc[hr][:, half * QB + fstart: (half + 1) * QB],
                                        KT[b0:b0 + DK, hp, th * P:(th + 1) * P],
                                        QT[b0:b0 + DK, hp, fstart:QB],
                                        start=True, stop=True)
                                if fstart or th == 4 * j:  # diagonal square
                                    for hr in range(2):
                                        dsl = sc[hr][:, half * QB + fstart:
                                                     half * QB + fstart + P]
                                        nc.vector.tensor_tensor(
                                            dsl, dsl, maskd[:], OP.add)
                            diag = fs[0] > 0 or 2 * tp == 4 * j
                            pr = [prp.tile([P, 2 * QB], FR, tag="probs", name=f"pr{_i}")
                                  for _i in range(2)]
                            for hr in range(2):
                                if diag:
                                    for half in range(2):
                                        a = half * QB + fs[half]
                                        nc.scalar.activation(
                                            pr[hr][:, a:(half + 1) * QB],
                                            sc[hr][:, a:(half + 1) * QB],
                                            AF.Exp, scale=0.125)
                                else:
                                    nc.scalar.activation(
                                        pr[hr][:], sc[hr][:], AF.Exp, scale=0.125)
                            for half in range(2):
                                th = 2 * tp + half
                                fstart = fs[half]
                                for hr in range(2):
                                    h = 2 * hp + hr
                                    nc.tensor.matmul(
                                        po[hr][0:DK + 1, fstart:QB],
                                        VA[:, th, h * (DK + 1):(h + 1) * (DK + 1)],
                                        pr[hr][:, half * QB + fstart:(half + 1) * QB],
                                        start=(th == 0), stop=(th == nt - 1))
                        for hr in range(2):
                            rec = rcp.tile([1, QB], FR, tag="rc")
                            with nc.allow_low_precision(reason="softmax recip fp32r"):
                                nc.vector.reciprocal(rec[:], po[hr][DK:DK + 1, :])
                            pb = ps_x.tile([P, QB], F32, tag="ps_x", name="pb")
                            nc.tensor.matmul(
                                pb[0:DK, :], ones_sb[0:1, 0:DK], rec[:],
                                start=True, stop=True)
                            rb = rbp.tile([DK, QB], F32, tag="rb")
                            nc.vector.tensor_copy(rb[:], pb[0:DK, :])
                            nc.vector.tensor_tensor(
                                ON[hr * DK:(hr + 1) * DK, hp, :],
                                po[hr][0:DK, :], rb[:], OP.mult)

                    for st in range(4):
                        y = yop.tile([P, D], F32, tag="yout")
                        for ob in range(2):
                            py = ps_x.tile([P, QB], F32, tag="ps_x")
                            for dt_i in range(NOT_):
                                nc.tensor.matmul(
                                    py[:], ON[:, dt_i, st * P:(st + 1) * P],
                                    wo_sb[:, dt_i, ob * QB:(ob + 1) * QB],
                                    start=(dt_i == 0), stop=(dt_i == NOT_ - 1))
                            nc.vector.tensor_tensor(
                                y[:, ob * QB:(ob + 1) * QB], py[:],
                                bob_sb[:, ob * QB:(ob + 1) * QB], OP.add)
                        nc.sync.dma_start(
                            out[j * QB + st * P: j * QB + (st + 1) * P, :], y[:])

            if loop_n is not None:
                with tc.For_i(0, loop_n, 1):
                    body()
            else:
                body()

    nc.compile()
    return nc


def make_in_maps(inputs):
    """Full inputs dict -> per-core in_maps (list of 8)."""
    query = np.asarray(inputs["query"], dtype=np.float32)
    key = np.asarray(inputs["key"], dtype=np.float32)
    value = np.asarray(inputs["value"], dtype=np.float32)
    W_q = np.asarray(inputs["W_q"], dtype=np.float32)
    W_k = np.asarray(inputs["W_k"], dtype=np.float32)
    W_v = np.asarray(inputs["W_v"], dtype=np.float32)
    W_o = np.asarray(inputs["W_o"], dtype=np.float32)
    b_q = np.asarray(inputs["b_q"], dtype=np.float32)
    b_k = np.asarray(inputs["b_k"], dtype=np.float32)
    b_v = np.asarray(inputs["b_v"], dtype=np.float32)
    b_o = np.asarray(inputs["b_o"], dtype=np.float32)

    ident = np.eye(P, dtype=np.float32)
    io = np.arange(P)
    maskd = np.where(io[None, :] >= io[:, None], 0.0, MASK_VAL).astype(np.float32)
    ones = np.ones((P, DK), dtype=np.float32)

    in_maps = []
    for c in range(8):
        b, hh = c // 2, c % 2
        sl = slice(hh * DH, (hh + 1) * DH)
        in_maps.append({
            "xq": np.ascontiguousarray(query[b]),
            "xk": np.ascontiguousarray(key[b]),
            "xv": np.ascontiguousarray(value[b]),
            "wqT": np.ascontiguousarray(W_q[sl, :].T),
            "wkT": np.ascontiguousarray(W_k[sl, :].T),
            "wvT": np.ascontiguousarray(W_v[sl, :].T),
            "woT": np.ascontiguousarray(W_o[:, sl].T),
            "bq": np.ascontiguousarray(b_q[sl]),
            "bk": np.ascontiguousarray(b_k[sl]),
            "bvb": np.tile(b_v[sl][None, :], (P, 1)).astype(np.float32),
            "bob": np.tile((0.5 * b_o)[None, :], (P, 1)).astype(np.float32),
            "ident": ident,
            "maskd": maskd,
            "ones": ones,
        })
    return in_maps


_nc_cache = {}


def get_nc(loop_n=None):
    if loop_n not in _nc_cache:
        _nc_cache[loop_n] = build_nc(loop_n)
    return _nc_cache[loop_n]


def kernel(**inputs) -> np.ndarray:
    nc = get_nc()
    in_maps = make_in_maps(inputs)
    res = run_bass_kernel_spmd(nc, in_maps, core_ids=list(range(8)))
    outs = [r["out"] for r in res.results]
    full = np.empty((B, S, D), dtype=np.float32)
    for b in range(B):
        full[b] = outs[2 * b] + outs[2 * b + 1]
    return full


# revision 6
# speedup vs baseline: 1.0208x; 1.0208x over previous
"""Multi-head causal attention (B=4, S=2048, D=1024, H=16) on 8 TRN2 NeuronCores.

Sharding: core c -> (batch b = c//2, head-half hh = c%2). Each core computes
8 heads (a 512-wide feature slice) for one batch element, including its
partial W_o projection; the host sums the two partials per batch.

Per-core kernel (all matmuls float32r, full-rate at N>=256):
  phase 1: PE-transpose key/value s-blocks to feature-major, project to
           K^T [512,2048] (feature-major) and V_aug [2048, 8x(64+1)] (natural,
           with a ones column per head for the softmax denominator).
  phase 2 (per 512-wide q-block): transpose+project Q^T; per head, scores^T
           tiles [k=128, q=512] via QK^T (2 heads row-packed in the PE array),
           causal mask on diagonal squares, exp on ACT (scale=1/8),
           PV matmul with lhsT=[V|1] accumulating unnormalized out^T and the
           denominator row in one PSUM tile; normalize via DVE reciprocal +
           K=1 ones-broadcast matmul; W_o partial projection; DMA out.
"""

import numpy as np

import concourse.bass as bass
import concourse.mybir as mybir
import concourse.tile as tile
from concourse import bacc
from concourse.bass_utils import run_bass_kernel_spmd

P = 128
FR = mybir.dt.float32r
F32 = mybir.dt.float32
AF = mybir.ActivationFunctionType
OP = mybir.AluOpType

B, S, D, H = 4, 2048, 1024, 16
DK = D // H            # 64
DH = D // 2            # 512: per-core feature slice (8 heads)
NH = DH // DK          # 8 heads per core
QB = 512               # q-block width
NJ = S // QB           # 4 q-blocks
NST = S // P           # 16 s-tiles
NDT = D // P           # 8 din tiles
NOT_ = DH // P         # 4 dout tiles (per-core slice)
MASK_VAL = -1e30


def build_nc(loop_n=None):
    nc = bacc.Bacc(None, target_bir_lowering=False)

    xq = nc.dram_tensor("xq", [S, D], FR, kind="ExternalInput")
    xk = nc.dram_tensor("xk", [S, D], FR, kind="ExternalInput")
    xv = nc.dram_tensor("xv", [S, D], FR, kind="ExternalInput")
    wqT = nc.dram_tensor("wqT", [D, DH], FR, kind="ExternalInput")
    wkT = nc.dram_tensor("wkT", [D, DH], FR, kind="ExternalInput")
    wvT = nc.dram_tensor("wvT", [D, DH], FR, kind="ExternalInput")
    woT = nc.dram_tensor("woT", [DH, D], FR, kind="ExternalInput")
    bq = nc.dram_tensor("bq", [DH], F32, kind="ExternalInput")
    bk = nc.dram_tensor("bk", [DH], F32, kind="ExternalInput")
    bvb = nc.dram_tensor("bvb", [P, DH], F32, kind="ExternalInput")
    bob = nc.dram_tensor("bob", [P, D], F32, kind="ExternalInput")
    ident_d = nc.dram_tensor("ident", [P, P], FR, kind="ExternalInput")
    maskd_d = nc.dram_tensor("maskd", [P, P], F32, kind="ExternalInput")
    ones_d = nc.dram_tensor("ones", [P, DK], FR, kind="ExternalInput")
    out = nc.dram_tensor("out", [S, D], F32, kind="ExternalOutput")

    with tile.TileContext(nc) as tc:
        with (
            tc.tile_pool(name="cst", bufs=1) as cst,
            tc.tile_pool(name="wt", bufs=2) as wtp,
            tc.tile_pool(name="big", bufs=1) as big,
            tc.tile_pool(name="xT", bufs=2) as xTp,
            tc.tile_pool(name="xnat", bufs=5) as xnp,
            tc.tile_pool(name="qt", bufs=1) as qtp,
            tc.tile_pool(name="probs", bufs=2) as prp,
            tc.tile_pool(name="onrm", bufs=1) as onp,
            tc.tile_pool(name="yout", bufs=2) as yop,
            tc.tile_pool(name="rc", bufs=2) as rcp,
            tc.tile_pool(name="rb", bufs=2) as rbp,
            tc.tile_pool(name="ps_s", bufs=2, space="PSUM") as ps_s,
            tc.tile_pool(name="ps_o", bufs=2, space="PSUM") as ps_o,
            tc.tile_pool(name="ps_x", bufs=2, space="PSUM") as ps_x,
        ):
            def body():
                ident = cst.tile([P, P], FR, tag="ident")
                nc.sync.dma_start(ident[:], ident_d[:])
                maskd = cst.tile([P, P], F32, tag="maskd")
                nc.sync.dma_start(maskd[:], maskd_d[:])
                ones_sb = cst.tile([P, DK], FR, tag="ones")
                nc.sync.dma_start(ones_sb[:], ones_d[:])
                bq_sb = cst.tile([P, NOT_], F32, tag="bq")
                nc.sync.dma_start(bq_sb[:], bq.rearrange("(o p) -> p o", p=P))
                bk_sb = cst.tile([P, NOT_], F32, tag="bk")
                nc.sync.dma_start(bk_sb[:], bk.rearrange("(o p) -> p o", p=P))
                bvb_sb = cst.tile([P, DH], F32, tag="bvb")
                nc.sync.dma_start(bvb_sb[:], bvb[:])
                bob_sb = cst.tile([P, D], F32, tag="bob")
                nc.sync.dma_start(bob_sb[:], bob[:])

                wk_sb = wtp.tile([P, NDT, DH], FR, tag="wt")
                nc.sync.dma_start(wk_sb[:], wkT.rearrange("(o p) f -> p o f", p=P))
                wv_sb = wtp.tile([P, NDT, DH], FR, tag="wt")
                nc.sync.dma_start(wv_sb[:], wvT.rearrange("(o p) f -> p o f", p=P))

                # persistent: K^T (feature-major) and V_aug (natural + ones col)
                KT = big.tile([P, NOT_, S], FR, tag="KT")          # 32KB/part
                VA = big.tile([P, NST, NH * (DK + 1)], FR, tag="VA")  # 32.5KB/part

                def transpose_block(x_dram, sb, copy_eng="scalar"):
                    """DMA s-block sb of x (natural) and PE-transpose to
                    feature-major xt [128, NDT, 512]. Returns the xT tile."""
                    xt = xTp.tile([P, NDT, QB], FR, tag="xT")
                    nats = []
                    for st in range(4):
                        xn = xnp.tile([P, D], FR, tag="xnat")
                        nc.sync.dma_start(
                            xn[:], x_dram[sb * QB + st * P: sb * QB + (st + 1) * P, :])
                        nats.append(xn)
                    for dt_i in range(NDT):
                        pt = ps_x.tile([P, QB], FR, tag="ps_x")
                        for st in range(4):
                            nc.tensor.transpose(
                                pt[:, st * P:(st + 1) * P],
                                nats[st][:, dt_i * P:(dt_i + 1) * P], ident[:])
                        if copy_eng == "scalar":
                            nc.scalar.copy(xt[:, dt_i, :], pt[:])
                        else:
                            nc.vector.tensor_copy(xt[:, dt_i, :], pt[:])
                    return xt

                # ---- phase 1: K^T and V_aug ----
                for sb in range(NJ):
                    kt_x = transpose_block(xk, sb)
                    for ot in range(NOT_):
                        pk = ps_x.tile([P, QB], F32, tag="ps_x")
                        for dt_i in range(NDT):
                            nc.tensor.matmul(
                                pk[:], wk_sb[:, dt_i, ot * P:(ot + 1) * P],
                                kt_x[:, dt_i, :],
                                start=(dt_i == 0), stop=(dt_i == NDT - 1))
                        nc.scalar.activation(
                            KT[:, ot, sb * QB:(sb + 1) * QB], pk[:],
                            AF.Identity, bias=bk_sb[:, ot:ot + 1])
                    vt_x = transpose_block(xv, sb)
                    for st in range(4):
                        stg = sb * 4 + st
                        pv = ps_x.tile([P, QB], F32, tag="ps_x")
                        for dt_i in range(NDT):
                            nc.tensor.matmul(
                                pv[:], vt_x[:, dt_i, st * P:(st + 1) * P],
                                wv_sb[:, dt_i, :],
                                start=(dt_i == 0), stop=(dt_i == NDT - 1))
                        va_row = VA[:, stg, :].rearrange("p (h e) -> p h e", e=DK + 1)
                        nc.vector.tensor_tensor(
                            va_row[:, :, 0:DK],
                            pv[:].rearrange("p (h e) -> p h e", e=DK),
                            bvb_sb[:].rearrange("p (h e) -> p h e", e=DK),
                            OP.add)
                        nc.sync.dma_start(va_row[:, :, DK], ones_d[:, 0:NH])

                wq_sb = wtp.tile([P, NDT, DH], FR, tag="wt")
                nc.sync.dma_start(wq_sb[:], wqT.rearrange("(o p) f -> p o f", p=P))
                wo_sb = wtp.tile([P, NOT_, D], FR, tag="wt")
                nc.sync.dma_start(wo_sb[:], woT.rearrange("(o p) f -> p o f", p=P))

                # ---- phase 2: per q-block ----
                for j in range(NJ):
                    qt_x = transpose_block(xq, j, copy_eng="vector")
                    QT = qtp.tile([P, NOT_, QB], FR, tag="qt")
                    for ot in range(NOT_):
                        pq = ps_x.tile([P, QB], F32, tag="ps_x")
                        for dt_i in range(NDT):
                            nc.tensor.matmul(
                                pq[:], wq_sb[:, dt_i, ot * P:(ot + 1) * P],
                                qt_x[:, dt_i, :],
                                start=(dt_i == 0), stop=(dt_i == NDT - 1))
                        nc.vector.tensor_scalar_add(
                            QT[:, ot, :], pq[:], bq_sb[:, ot:ot + 1])

                    ON = onp.tile([P, NOT_, QB], FR, tag="onrm")
                    nt = 4 * j + 4          # k-tiles for this q-block
                    for hp in range(NOT_):
                        po = [ps_o.tile([P, QB], F32, tag="ps_o", name=f"po{_i}") for _i in range(2)]
                        for tp in range(nt // 2):
                            sc = [ps_s.tile([P, 2 * QB], F32, tag="ps_s", name=f"sc{_i}")
                                  for _i in range(2)]
                            fs = []
                            for half in range(2):
                                th = 2 * tp + half
                                fstart = max(0, P * (th - 4 * j))
                                fs.append(fstart)
                                for hr in range(2):
                                    b0 = hr * DK
                                    nc.tensor.matmul(
                                        sc[hr][:, half * QB + fstart: (half + 1) * QB],
                                        KT[b0:b0 + DK, hp, th * P:(th + 1) * P],
                                        QT[b0:b0 + DK, hp, fstart:QB],
                                        start=True, stop=True)
                                if fstart or th == 4 * j:  # diagonal square
                                    for hr in range(2):
                                        dsl = sc[hr][:, half * QB + fstart:
                                                     half * QB + fstart + P]
                                        nc.vector.tensor_tensor(
                                            dsl, dsl, maskd[:], OP.add)
                            diag = fs[0] > 0 or 2 * tp == 4 * j
                            pr = [prp.tile([P, 2 * QB], FR, tag="probs", name=f"pr{_i}")
                                  for _i in range(2)]
                            for hr in range(2):
                                if diag:
                                    for half in range(2):
                                        a = half * QB + fs[half]
                                        nc.scalar.activation(
                                            pr[hr][:, a:(half + 1) * QB],
                                            sc[hr][:, a:(half + 1) * QB],
                                            AF.Exp, scale=0.125)
                                else:
                                    nc.scalar.activation(
                                        pr[hr][:], sc[hr][:], AF.Exp, scale=0.125)
                            for half in range(2):
                                th = 2 * tp + half
                                fstart = fs[half]
                                for hr in range(2):
                                    h = 2 * hp + hr
                                    nc.tensor.matmul(
                                        po[hr][0:DK + 1, fstart:QB],
                                        VA[:, th, h * (DK + 1):(h + 1) * (DK + 1)],
                                        pr[hr][:, half * QB + fstart:(half + 1) * QB],
                                        start=(th == 0), stop=(th == nt - 1))
                        for hr in range(2):
                            rec = rcp.tile([1, QB], FR, tag="rc")
                            with nc.allow_low_precision(reason="softmax recip fp32r"):
                                nc.vector.reciprocal(rec[:], po[hr][DK:DK + 1, :])
                            pb = ps_x.tile([P, QB], F32, tag="ps_x", name="pb")
                            nc.tensor.matmul(
                                pb[0:DK, :], ones_sb[0:1, 0:DK], rec[:],
                                start=True, stop=True)
                            rb = rbp.tile([DK, QB], F32, tag="rb")
                            nc.vector.tensor_copy(rb[:], pb[0:DK, :])
                            nc.vector.tensor_tensor(
                                ON[hr * DK:(hr + 1) * DK, hp, :],
                                po[hr][0:DK, :], rb[:], OP.mult)

                    for st in range(4):
                        y = yop.tile([P, D], F32, tag="yout")
                        for ob in range(2):
                            py = ps_x.tile([P, QB], F32, tag="ps_x")
                            for dt_i in range(NOT_):
                                nc.tensor.matmul(
                                    py[:], ON[:, dt_i, st * P:(st + 1) * P],
                                    wo_sb[:, dt_i, ob * QB:(ob + 1) * QB],
                                    start=(dt_i == 0), stop=(dt_i == NOT_ - 1))
                            nc.vector.tensor_tensor(
                                y[:, ob * QB:(ob + 1) * QB], py[:],
                                bob_sb[:, ob * QB:(ob + 1) * QB], OP.add)
                        nc.sync.dma_start(
                            out[j * QB + st * P: j * QB + (st + 1) * P, :], y[:])

            if loop_n is not None:
                with tc.For_i(0, loop_n, 1):
                    body()
            else:
                body()

    nc.compile()
    return nc


def make_in_maps(inputs):
    """Full inputs dict -> per-core in_maps (list of 8)."""
    query = np.asarray(inputs["query"], dtype=np.float32)
    key = np.asarray(inputs["key"], dtype=np.float32)
    value = np.asarray(inputs["value"], dtype=np.float32)
    W_q = np.asarray(inputs["W_q"], dtype=np.float32)
    W_k = np.asarray(inputs["W_k"], dtype=np.float32)
    W_v = np.asarray(inputs["W_v"], dtype=np.float32)
    W_o = np.asarray(inputs["W_o"], dtype=np.float32)
    b_q = np.asarray(inputs["b_q"], dtype=np.float32)
    b_k = np.asarray(inputs["b_k"], dtype=np.float32)
    b_v = np.asarray(inputs["b_v"], dtype=np.float32)
    b_o = np.asarray(inputs["b_o"], dtype=np.float32)

    ident = np.eye(P, dtype=np.float32)
    io = np.arange(P)
    maskd = np.where(io[None, :] >= io[:, None], 0.0, MASK_VAL).astype(np.float32)
    ones = np.ones((P, DK), dtype=np.float32)

    in_maps = []
    for c in range(8):
        b, hh = c // 2, c % 2
        sl = slice(hh * DH, (hh + 1) * DH)
        in_maps.append({
            "xq": np.ascontiguousarray(query[b]),
            "xk": np.ascontiguousarray(key[b]),
            "xv": np.ascontiguousarray(value[b]),
            "wqT": np.ascontiguousarray(W_q[sl, :].T),
            "wkT": np.ascontiguousarray(W_k[sl, :].T),
            "wvT": np.ascontiguousarray(W_v[sl, :].T),
            "woT": np.ascontiguousarray(W_o[:, sl].T),
            "bq": np.ascontiguousarray(b_q[sl]),
            "bk": np.ascontiguousarray(b_k[sl]),
            "bvb": np.tile(b_v[sl][None, :], (P, 1)).astype(np.float32),
            "bob": np.tile((0.5 * b_o)[None, :], (P, 1)).astype(np.float32),
            "ident": ident,
            "maskd": maskd,
            "ones": ones,
        })
    return in_maps


_nc_cache = {}


def get_nc(loop_n=None):
    if loop_n not in _nc_cache:
        _nc_cache[loop_n] = build_nc(loop_n)
    return _nc_cache[loop_n]


def kernel(**inputs) -> np.ndarray:
    nc = get_nc()
    in_maps = make_in_maps(inputs)
    res = run_bass_kernel_spmd(nc, in_maps, core_ids=list(range(8)))
    outs = [r["out"] for r in res.results]
    full = np.empty((B, S, D), dtype=np.float32)
    for b in range(B):
        full[b] = outs[2 * b] + outs[2 * b + 1]
    return full


# revision 11
# speedup vs baseline: 1.3960x; 1.3676x over previous
"""Multi-head causal attention (B=4, S=2048, D=1024, H=16) on 8 TRN2 NeuronCores.

Sharding: core c -> (batch b = c//2, head-half hh = c%2). Each core computes
8 heads (a 512-wide feature slice) for one batch element, including its
partial W_o projection; the host sums the two partials per batch.

Per-core kernel (all matmuls float32r, full-rate at N>=256):
  phase 1: PE-transpose key/value s-blocks to feature-major, project to
           K^T [512,2048] (feature-major) and V_aug [2048, 8x(64+1)] (natural,
           with a ones column per head for the softmax denominator).
  phase 2 (per 512-wide q-block): transpose+project Q^T; per head, scores^T
           tiles [k=128, q=512] via QK^T (2 heads row-packed in the PE array),
           causal mask on diagonal squares, exp on ACT (scale=1/8),
           PV matmul with lhsT=[V|1] accumulating unnormalized out^T and the
           denominator row in one PSUM tile; normalize via DVE reciprocal +
           K=1 ones-broadcast matmul; W_o partial projection; DMA out.
"""

import numpy as np

import concourse.bass as bass
import concourse.mybir as mybir
import concourse.tile as tile
from concourse import bacc
from concourse.bass_utils import run_bass_kernel_spmd

P = 128
FR = mybir.dt.float32r
F32 = mybir.dt.float32
AF = mybir.ActivationFunctionType
OP = mybir.AluOpType

B, S, D, H = 4, 2048, 1024, 16
DK = D // H            # 64
DH = D // 2            # 512: per-core feature slice (8 heads)
NH = DH // DK          # 8 heads per core
QB = 512               # q-block width
NJ = S // QB           # 4 q-blocks
NST = S // P           # 16 s-tiles
NDT = D // P           # 8 din tiles
NOT_ = DH // P         # 4 dout tiles (per-core slice)
MASK_VAL = -1e30


def build_nc(loop_n=None):
    nc = bacc.Bacc(None, target_bir_lowering=False)

    xq = nc.dram_tensor("xq", [S, D], FR, kind="ExternalInput")
    xk = nc.dram_tensor("xk", [S, D], FR, kind="ExternalInput")
    xv = nc.dram_tensor("xv", [S, D], FR, kind="ExternalInput")
    wqT = nc.dram_tensor("wqT", [D, DH], FR, kind="ExternalInput")
    wkT = nc.dram_tensor("wkT", [D, DH], FR, kind="ExternalInput")
    wvT = nc.dram_tensor("wvT", [D, DH], FR, kind="ExternalInput")
    woT = nc.dram_tensor("woT", [DH, D], FR, kind="ExternalInput")
    bq = nc.dram_tensor("bq", [DH], F32, kind="ExternalInput")
    bk = nc.dram_tensor("bk", [DH], F32, kind="ExternalInput")
    bvb = nc.dram_tensor("bvb", [P, DH], F32, kind="ExternalInput")
    bob = nc.dram_tensor("bob", [P, D], F32, kind="ExternalInput")
    ident_d = nc.dram_tensor("ident", [P, P], FR, kind="ExternalInput")
    maskd_d = nc.dram_tensor("maskd", [P, P], F32, kind="ExternalInput")
    ones_d = nc.dram_tensor("ones", [P, DK], FR, kind="ExternalInput")
    out = nc.dram_tensor("out", [S, D], F32, kind="ExternalOutput")

    with tile.TileContext(nc) as tc:
        with (
            tc.tile_pool(name="cst", bufs=1) as cst,
            tc.tile_pool(name="wt", bufs=2) as wtp,
            tc.tile_pool(name="big", bufs=1) as big,
            tc.tile_pool(name="xT", bufs=1) as xTp,
            tc.tile_pool(name="xnat", bufs=5) as xnp,
            tc.tile_pool(name="qt", bufs=2) as qtp,
            tc.tile_pool(name="probs", bufs=2) as prp,
            tc.tile_pool(name="onrm", bufs=2) as onp,
            tc.tile_pool(name="yout", bufs=2) as yop,
            tc.tile_pool(name="rc", bufs=2) as rcp,
            tc.tile_pool(name="posb", bufs=2) as pop,
            tc.tile_pool(name="ps_s", bufs=2, space="PSUM") as ps_s,
            tc.tile_pool(name="ps_o", bufs=2, space="PSUM") as ps_o,
            tc.tile_pool(name="ps_x", bufs=2, space="PSUM") as ps_x,
        ):
            def body():
                ident = cst.tile([P, P], FR, tag="ident")
                nc.sync.dma_start(ident[:], ident_d[:])
                maskd = cst.tile([P, P], F32, tag="maskd")
                nc.sync.dma_start(maskd[:], maskd_d[:])
                ones_sb = cst.tile([P, DK], FR, tag="ones")
                nc.sync.dma_start(ones_sb[:], ones_d[:])
                bq_sb = cst.tile([P, NOT_], F32, tag="bq")
                nc.sync.dma_start(bq_sb[:], bq.rearrange("(o p) -> p o", p=P))
                bk_sb = cst.tile([P, NOT_], F32, tag="bk")
                nc.sync.dma_start(bk_sb[:], bk.rearrange("(o p) -> p o", p=P))
                bvb_sb = cst.tile([P, DH], F32, tag="bvb")
                nc.sync.dma_start(bvb_sb[:], bvb[:])
                bob_sb = cst.tile([P, D], F32, tag="bob")
                nc.sync.dma_start(bob_sb[:], bob[:])

                wk_sb = wtp.tile([P, NDT, DH], FR, tag="wt")
                nc.sync.dma_start(wk_sb[:], wkT.rearrange("(o p) f -> p o f", p=P))
                wv_sb = wtp.tile([P, NDT, DH], FR, tag="wt")
                nc.sync.dma_start(wv_sb[:], wvT.rearrange("(o p) f -> p o f", p=P))

                # persistent: K^T (feature-major) and V_aug (natural + ones col)
                KT = big.tile([P, NOT_, S], FR, tag="KT")          # 32KB/part
                VA = big.tile([P, NST, NH * (DK + 1)], FR, tag="VA")  # 32.5KB/part

                def transpose_block(x_dram, sb, copy_eng="scalar"):
                    """DMA s-block sb of x (natural) and PE-transpose to
                    feature-major xt [128, NDT, 512]. Returns the xT tile."""
                    xt = xTp.tile([P, NDT, QB], FR, tag="xT")
                    nats = []
                    for st in range(4):
                        xn = xnp.tile([P, D], FR, tag="xnat")
                        nc.sync.dma_start(
                            xn[:], x_dram[sb * QB + st * P: sb * QB + (st + 1) * P, :])
                        nats.append(xn)
                    for dt_i in range(NDT):
                        pt = ps_x.tile([P, QB], FR, tag="ps_x")
                        for st in range(4):
                            nc.tensor.transpose(
                                pt[:, st * P:(st + 1) * P],
                                nats[st][:, dt_i * P:(dt_i + 1) * P], ident[:])
                        if copy_eng == "scalar":
                            nc.scalar.copy(xt[:, dt_i, :], pt[:])
                        else:
                            nc.vector.tensor_copy(xt[:, dt_i, :], pt[:])
                    return xt

                # ---- phase 1: K^T and V_aug ----
                for sb in range(NJ):
                    kt_x = transpose_block(xk, sb)
                    for ot in range(NOT_):
                        pk = ps_x.tile([P, QB], F32, tag="ps_x")
                        for dt_i in range(NDT):
                            nc.tensor.matmul(
                                pk[:], wk_sb[:, dt_i, ot * P:(ot + 1) * P],
                                kt_x[:, dt_i, :],
                                start=(dt_i == 0), stop=(dt_i == NDT - 1))
                        nc.scalar.activation(
                            KT[:, ot, sb * QB:(sb + 1) * QB], pk[:],
                            AF.Identity, bias=bk_sb[:, ot:ot + 1])
                    vt_x = transpose_block(xv, sb)
                    for st in range(4):
                        stg = sb * 4 + st
                        pv = ps_x.tile([P, QB], F32, tag="ps_x")
                        for dt_i in range(NDT):
                            nc.tensor.matmul(
                                pv[:], vt_x[:, dt_i, st * P:(st + 1) * P],
                                wv_sb[:, dt_i, :],
                                start=(dt_i == 0), stop=(dt_i == NDT - 1))
                        va_row = VA[:, stg, :].rearrange("p (h e) -> p h e", e=DK + 1)
                        nc.vector.tensor_tensor(
                            va_row[:, :, 0:DK],
                            pv[:].rearrange("p (h e) -> p h e", e=DK),
                            bvb_sb[:].rearrange("p (h e) -> p h e", e=DK),
                            OP.add)
                        nc.sync.dma_start(va_row[:, :, DK], ones_d[:, 0:NH])

                wq_sb = wtp.tile([P, NDT, DH], FR, tag="wt")
                nc.sync.dma_start(wq_sb[:], wqT.rearrange("(o p) f -> p o f", p=P))
                wo_sb = wtp.tile([P, NOT_, D], FR, tag="wt")
                nc.sync.dma_start(wo_sb[:], woT.rearrange("(o p) f -> p o f", p=P))

                # ---- phase 2: per q-block, software-pipelined ----
                QTs, ONs = {}, {}

                def gen_qlin(j):
                    """Generator: transposes + Q-projection for block j,
                    yielding between small PE chunks so the emission (and so
                    each engine's program order) interleaves with attention."""
                    xt = xTp.tile([P, NDT, QB], FR, tag="xT", name="xt")
                    nats = []
                    for st in range(4):
                        xn = xnp.tile([P, D], FR, tag="xnat", name="xn")
                        nc.sync.dma_start(
                            xn[:], xq[j * QB + st * P: j * QB + (st + 1) * P, :])
                        nats.append(xn)
                    yield
                    for dt_i in range(NDT):
                        pt = ps_x.tile([P, QB], FR, tag="ps_x", name="pt")
                        for st in range(4):
                            nc.tensor.transpose(
                                pt[:, st * P:(st + 1) * P],
                                nats[st][:, dt_i * P:(dt_i + 1) * P], ident[:])
                        nc.vector.tensor_copy(xt[:, dt_i, :], pt[:])
                        yield
                    QT = qtp.tile([P, NOT_, QB], FR, tag="qt", name="QT")
                    QTs[j] = QT
                    for ot in range(NOT_):
                        pq = ps_x.tile([P, QB], F32, tag="ps_x", name="pq")
                        for dt_i in range(NDT):
                            nc.tensor.matmul(
                                pq[:], wq_sb[:, dt_i, ot * P:(ot + 1) * P],
                                xt[:, dt_i, :],
                                start=(dt_i == 0), stop=(dt_i == NDT - 1))
                        nc.vector.tensor_scalar_add(
                            QT[:, ot, :], pq[:], bq_sb[:, ot:ot + 1])
                        yield

                def gen_wo(j):
                    """Generator: W_o partial projection + output DMA for
                    block j, in per-psum-group chunks."""
                    ON = ONs[j]
                    for st in range(4):
                        y = yop.tile([P, D], F32, tag="yout", name="y")
                        for ob in range(2):
                            py = ps_x.tile([P, QB], F32, tag="ps_x", name="py")
                            for dt_i in range(NOT_):
                                nc.tensor.matmul(
                                    py[:], ON[:, dt_i, st * P:(st + 1) * P],
                                    wo_sb[:, dt_i, ob * QB:(ob + 1) * QB],
                                    start=(dt_i == 0), stop=(dt_i == NOT_ - 1))
                            nc.vector.tensor_tensor(
                                y[:, ob * QB:(ob + 1) * QB], py[:],
                                bob_sb[:, ob * QB:(ob + 1) * QB], OP.add)
                            yield
                        nc.sync.dma_start(
                            out[j * QB + st * P: j * QB + (st + 1) * P, :], y[:])

                def advance(gens):
                    for g in list(gens):
                        try:
                            next(g)
                            return
                        except StopIteration:
                            gens.remove(g)

                g0 = gen_qlin(0)
                for _ in g0:
                    pass
                for j in range(NJ):
                    gens = []
                    if j + 1 < NJ:
                        gens.append(gen_qlin(j + 1))
                    if j > 0:
                        gens.append(gen_wo(j - 1))
                    QT = QTs[j]
                    ON = onp.tile([P, NOT_, QB], FR, tag="onrm", name="ON")
                    ONs[j] = ON
                    nt = 4 * j + 4          # k-tiles for this q-block
                    for hp in range(NOT_):
                        po = [ps_o.tile([P, QB], F32, tag="ps_o", name=f"po{_i}") for _i in range(2)]
                        for tp in range(nt // 2):
                            sc = [ps_s.tile([P, 2 * QB], F32, tag="ps_s", name=f"sc{_i}")
                                  for _i in range(2)]
                            fs = []
                            for half in range(2):
                                th = 2 * tp + half
                                fstart = max(0, P * (th - 4 * j))
                                fs.append(fstart)
                                for hr in range(2):
                                    b0 = hr * DK
                                    nc.tensor.matmul(
                                        sc[hr][:, half * QB + fstart: (half + 1) * QB],
                                        KT[b0:b0 + DK, hp, th * P:(th + 1) * P],
                                        QT[b0:b0 + DK, hp, fstart:QB],
                                        start=True, stop=True)
                                if fstart or th == 4 * j:  # diagonal square
                                    for hr in range(2):
                                        dsl = sc[hr][:, half * QB + fstart:
                                                     half * QB + fstart + P]
                                        nc.vector.tensor_tensor(
                                            dsl, dsl, maskd[:], OP.add)
                            diag = fs[0] > 0 or 2 * tp == 4 * j
                            pr = [prp.tile([P, 2 * QB], FR, tag="probs", name=f"pr{_i}")
                                  for _i in range(2)]
                            for hr in range(2):
                                if diag:
                                    for half in range(2):
                                        a = half * QB + fs[half]
                                        nc.scalar.activation(
                                            pr[hr][:, a:(half + 1) * QB],
                                            sc[hr][:, a:(half + 1) * QB],
                                            AF.Exp, scale=0.125)
                                else:
                                    nc.scalar.activation(
                                        pr[hr][:], sc[hr][:], AF.Exp, scale=0.125)
                            for half in range(2):
                                th = 2 * tp + half
                                fstart = fs[half]
                                for hr in range(2):
                                    h = 2 * hp + hr
                                    nc.tensor.matmul(
                                        po[hr][0:DK + 1, fstart:QB],
                                        VA[:, th, h * (DK + 1):(h + 1) * (DK + 1)],
                                        pr[hr][:, half * QB + fstart:(half + 1) * QB],
                                        start=(th == 0), stop=(th == nt - 1))
                            advance(gens)
                        for hr in range(2):
                            posb = pop.tile([DK + 1, QB], F32, tag="posb")
                            nc.vector.tensor_copy(posb[:], po[hr][0:DK + 1, :])
                            rec = rcp.tile([1, QB], FR, tag="rc")
                            with nc.allow_low_precision(reason="softmax recip fp32r"):
                                nc.vector.reciprocal(rec[:], posb[DK:DK + 1, :])
                            pb = ps_x.tile([P, QB], F32, tag="ps_x", name="pb")
                            nc.tensor.matmul(
                                pb[0:DK, :], ones_sb[0:1, 0:DK], rec[:],
                                start=True, stop=True)
                            nc.vector.tensor_tensor(
                                ON[hr * DK:(hr + 1) * DK, hp, :],
                                posb[0:DK, :], pb[0:DK, :], OP.mult)
                    for g in gens:
                        for _ in g:
                            pass
                for _ in gen_wo(NJ - 1):
                    pass

            if loop_n is not None:
                with tc.For_i(0, loop_n, 1):
                    body()
            else:
                body()

    nc.compile()
    return nc


def make_in_maps(inputs):
    """Full inputs dict -> per-core in_maps (list of 8)."""
    query = np.asarray(inputs["query"], dtype=np.float32)
    key = np.asarray(inputs["key"], dtype=np.float32)
    value = np.asarray(inputs["value"], dtype=np.float32)
    W_q = np.asarray(inputs["W_q"], dtype=np.float32)
    W_k = np.asarray(inputs["W_k"], dtype=np.float32)
    W_v = np.asarray(inputs["W_v"], dtype=np.float32)
    W_o = np.asarray(inputs["W_o"], dtype=np.float32)
    b_q = np.asarray(inputs["b_q"], dtype=np.float32)
    b_k = np.asarray(inputs["b_k"], dtype=np.float32)
    b_v = np.asarray(inputs["b_v"], dtype=np.float32)
    b_o = np.asarray(inputs["b_o"], dtype=np.float32)

    ident = np.eye(P, dtype=np.float32)
    io = np.arange(P)
    maskd = np.where(io[None, :] >= io[:, None], 0.0, MASK_VAL).astype(np.float32)
    ones = np.ones((P, DK), dtype=np.float32)

    in_maps = []
    for c in range(8):
        b, hh = c // 2, c % 2
        sl = slice(hh * DH, (hh + 1) * DH)
        in_maps.append({
            "xq": np.ascontiguousarray(query[b]),
            "xk": np.ascontiguousarray(key[b]),
            "xv": np.ascontiguousarray(value[b]),
            "wqT": np.ascontiguousarray(W_q[sl, :].T),
            "wkT": np.ascontiguousarray(W_k[sl, :].T),
            "wvT": np.ascontiguousarray(W_v[sl, :].T),
            "woT": np.ascontiguousarray(W_o[:, sl].T),
            "bq": np.ascontiguousarray(b_q[sl]),
            "bk": np.ascontiguousarray(b_k[sl]),
            "bvb": np.tile(b_v[sl][None, :], (P, 1)).astype(np.float32),
            "bob": np.tile((0.5 * b_o)[None, :], (P, 1)).astype(np.float32),
            "ident": ident,
            "maskd": maskd,
            "ones": ones,
        })
    return in_maps


_nc_cache = {}


def get_nc(loop_n=None):
    if loop_n not in _nc_cache:
        _nc_cache[loop_n] = build_nc(loop_n)
    return _nc_cache[loop_n]


def kernel(**inputs) -> np.ndarray:
    nc = get_nc()
    in_maps = make_in_maps(inputs)
    res = run_bass_kernel_spmd(nc, in_maps, core_ids=list(range(8)))
    outs = [r["out"] for r in res.results]
    full = np.empty((B, S, D), dtype=np.float32)
    for b in range(B):
        full[b] = outs[2 * b] + outs[2 * b + 1]
    return full
